# revision 1
# baseline (speedup 1.0000x reference)
"""Bass/Trainium2 kernel for nn_BigramLanguageModel.

Strategy (8 NeuronCores, single SPMD launch, no collectives):
  - The small 3-layer transformer (C=384, T=1024, B=2) is replicated on all
    8 cores (on-chip collectives have a ~10us/ring-step latency floor that
    dwarfs the work they would save).
  - The LM head (C x 50257 GEMM -> 412 MB of logits) dominates compute and
    output bandwidth; it is sharded over the vocab dim: core j computes
    logits[:, 6400*j : 6400*(j+1)] against a padded (384, 51200) Wlm, and
    the host concatenates the shards. Wlm is prefetched under the last MLP.
  - All GEMMs run as float32r (full-rate fp32, ~13-bit mantissa, rel err
    ~1.5e-4 per GEMM). Residual stream h stays fp32 in SBUF all the way.
  - LayerNorm affine (g, b) is folded into the following projection weights
    host-side; 1/sqrt(HS) is folded into Wk; q/k biases are folded into the
    PSUM evacuations (per-partition scalar_tensor_tensor), b1 into the Relu
    evacuation; v/o/mlp2 biases ride as K=1 rank-1 PSUM-preload matmuls.
  - Softmax over the *query* axis (dim=-2 in the reference) is computed in
    the transposed score layout wT[k, t] so the reduction runs along the
    free axis: scores for one (head, key-chunk) land in a (128, 1024)
    two-bank PSUM tile and a single Exp activation with fused accum_out
    yields p and the denominator in one pass over the causally-valid range;
    1/denom is folded into v rows (per-partition scale).
  - The causal mask is a bf16 (-80)-triangle PSUM-preload matmul on the
    diagonal score block (bf16 avoids the fp32r small-N matmul penalty).
  - Head pairs share 128-partition tiles; attention p@v accumulates in
    (64, 1024) PSUM tiles with the two heads in separate banks.
  - PSUM pools are phase-scoped (max 8 banks at any point); evacuations are
    split DVE/ACT by phase load.
"""

import sys

sys.path.insert(0, "/opt/trn_rl_repo")

import numpy as np

import concourse.bass as bass
import concourse.mybir as mybir
import concourse.tile as tile
from concourse import bacc
from concourse import bass_utils

F32 = mybir.dt.float32
BF16 = mybir.dt.bfloat16
F32R = mybir.dt.float32r
I32 = mybir.dt.int32
AF = mybir.ActivationFunctionType
ALU = mybir.AluOpType

V, C, T, H, HS, NL, B = 50257, 384, 1024, 6, 64, 3, 2
P = 128
N = B * T                  # 2048 tokens
NT = N // P                # 16 token chunks
NC3 = C // P               # 3 channel chunks
NCORE = 8
VPAD = 51200               # padded vocab (8 * 6400)
VSH = VPAD // NCORE        # 6400 vocab columns per core
KC = T // P                # 8 key chunks per batch
TB = T // 512              # 2 query blocks of 512 per batch
NEG = -80.0                # mask bias (exp(-80) ~ 1.8e-35)

_CACHE: dict = {}
PHASES: list = []


def _mark(nc, label):
    PHASES.append((label, int(nc.next_id())))


def _valid_lo(kc, tb):
    """First valid query column (within the 512-wide block tb) for key chunk
    kc, or None if the whole block is masked. Valid means t >= 128*kc."""
    lo = 128 * kc - 512 * tb
    if lo >= 512:
        return None
    return max(lo, 0)


class _EvacSplit:
    """Weighted round-robin of PSUM->SBUF evacuation copies over DVE/ACT."""

    def __init__(self, nc, dve_share=2, act_share=1):
        self.nc = nc
        self.i = 0
        self.period = dve_share + act_share
        self.dve_share = dve_share

    def copy(self, out, in_):
        self.i += 1
        if self.i % self.period < self.dve_share:
            self.nc.vector.tensor_copy(out, in_)
        else:
            self.nc.scalar.copy(out, in_)

    def copy_bias(self, out, in_, bias_col):
        """out = in_ + bias (bias per partition, (p,1) AP)."""
        self.i += 1
        if self.i % self.period < self.dve_share:
            self.nc.vector.scalar_tensor_tensor(
                out, in_, bias_col, out,
                op0=mybir.AluOpType.add, op1=mybir.AluOpType.bypass)
        else:
            self.nc.scalar.activation(out, in_, AF.Identity, bias=bias_col)


def _build(has_blm: bool):
    nc = bacc.Bacc("TRN2", target_bir_lowering=False, debug=False)

    # ---------------- DRAM I/O ----------------
    d_idx = nc.dram_tensor("idx", [N, 1], I32, kind="ExternalInput").ap()
    d_tok = nc.dram_tensor("tok_emb", [V, C], F32, kind="ExternalInput").ap()
    d_pos = nc.dram_tensor("pos", [N, C], F32, kind="ExternalInput").ap()
    d_wq = nc.dram_tensor("wq", [NL * C, C], F32R, kind="ExternalInput").ap()
    d_wk = nc.dram_tensor("wk", [NL * C, C], F32R, kind="ExternalInput").ap()
    d_wv = nc.dram_tensor("wv", [NL * C, C], F32R, kind="ExternalInput").ap()
    d_wo = nc.dram_tensor("wo", [NL * C, C], F32R, kind="ExternalInput").ap()
    d_w1 = nc.dram_tensor("w1", [NL * C, C], F32R, kind="ExternalInput").ap()
    d_w2 = nc.dram_tensor("w2", [NL * C, C], F32R, kind="ExternalInput").ap()
    d_bq = nc.dram_tensor("bq", [NL, C], F32R, kind="ExternalInput").ap()
    d_bk = nc.dram_tensor("bk", [NL, C], F32R, kind="ExternalInput").ap()
    d_bv = nc.dram_tensor("bv", [NL, C], F32R, kind="ExternalInput").ap()
    d_bo = nc.dram_tensor("bo", [NL, C], F32R, kind="ExternalInput").ap()
    d_b1 = nc.dram_tensor("b1", [NL, C], F32R, kind="ExternalInput").ap()
    d_b2 = nc.dram_tensor("b2", [NL, C], F32R, kind="ExternalInput").ap()
    d_ones = nc.dram_tensor("ones", [1, 512], F32R, kind="ExternalInput").ap()
    d_ident = nc.dram_tensor("ident", [P, P], F32, kind="ExternalInput").ap()
    d_identb = nc.dram_tensor("identb", [P, P], BF16, kind="ExternalInput").ap()
    d_trib = nc.dram_tensor("trib", [P, P], BF16, kind="ExternalInput").ap()
    d_bqkt = nc.dram_tensor("bqkt", [NL * C, 2], F32, kind="ExternalInput").ap()
    d_b1t = nc.dram_tensor("b1t", [NL * C, 1], F32, kind="ExternalInput").ap()
    d_wlm = nc.dram_tensor("wlm", [C, VSH], F32R, kind="ExternalInput").ap()
    if has_blm:
        d_blm = nc.dram_tensor("blm", [1, VSH], F32R, kind="ExternalInput").ap()
    d_out = nc.dram_tensor("logits", [N, VSH], F32, kind="ExternalOutput").ap()

    with tile.TileContext(nc) as tc:
        _emit(nc, tc, locals(), has_blm)
    nc.compile()
    return nc


def _emit(nc, tc, d, has_blm):
    from contextlib import ExitStack

    with ExitStack() as ctx:
        ev_lm = None
        hpool = ctx.enter_context(tc.tile_pool(name="hpool", bufs=NT))
        pers = ctx.enter_context(tc.tile_pool(name="pers", bufs=1))
        spool = ctx.enter_context(tc.tile_pool(name="spool", bufs=8))

        ev = _EvacSplit(nc)

        # ------------- constants -------------
        ones = pers.tile([1, 512], F32R, name="ones", tag="ones")
        ident = pers.tile([P, P], F32, name="ident", tag="ident")
        identb = pers.tile([P, P], BF16, name="identb", tag="identb")
        trib = pers.tile([P, P], BF16, name="trib", tag="trib")
        eps = pers.tile([P, 1], F32, name="eps", tag="eps")
        nc.sync.dma_start(ones[:], d["d_ones"][:])
        nc.sync.dma_start(ident[:], d["d_ident"][:])
        nc.sync.dma_start(identb[:], d["d_identb"][:])
        nc.sync.dma_start(trib[:], d["d_trib"][:])
        nc.vector.memset(eps[:], 1e-5)

        # ------------- embedding gather -------------
        _mark(nc, "embed")
        h = []  # 16 residual-stream tiles (128, 384) fp32, live whole kernel
        with tc.tile_pool(name="epool", bufs=3) as epool:
            for i in range(NT):
                idx_t = spool.tile([P, 1], I32, name="idx", tag="idx")
                nc.sync.dma_start(idx_t[:], d["d_idx"][i * P:(i + 1) * P, :])
                emb = epool.tile([P, C], F32, name="emb", tag="emb")
                nc.gpsimd.indirect_dma_start(
                    out=emb[:], out_offset=None, in_=d["d_tok"][:],
                    in_offset=bass.IndirectOffsetOnAxis(ap=idx_t[:, :1], axis=0),
                )
                pos_t = epool.tile([P, C], F32, name="pos", tag="pos")
                nc.sync.dma_start(pos_t[:], d["d_pos"][i * P:(i + 1) * P, :])
                h_i = hpool.tile([P, C], F32, name="h", tag="h")
                nc.vector.tensor_add(h_i[:], emb[:], pos_t[:])
                h.append(h_i)

        # ------------- layers -------------
        for l in range(NL):
            with ExitStack() as lctx:
                wpool = lctx.enter_context(
                    tc.tile_pool(name=f"wpool{l}", bufs=1))
                wq = [wpool.tile([P, C], F32R, name=f"wq{c}", tag=f"wq{c}")
                      for c in range(NC3)]
                wk = [wpool.tile([P, C], F32R, name=f"wk{c}", tag=f"wk{c}")
                      for c in range(NC3)]
                wv = [wpool.tile([P, C], F32R, name=f"wv{c}", tag=f"wv{c}")
                      for c in range(NC3)]
                wo = [wpool.tile([P, C], F32R, name=f"wo{c}", tag=f"wo{c}")
                      for c in range(NC3)]
                w1 = [wpool.tile([P, C], F32R, name=f"w1{c}", tag=f"w1{c}")
                      for c in range(NC3)]
                w2 = [wpool.tile([P, C], F32R, name=f"w2{c}", tag=f"w2{c}")
                      for c in range(NC3)]
                for c in range(NC3):
                    r0 = l * C + c * P
                    nc.sync.dma_start(wq[c][:], d["d_wq"][r0:r0 + P, :])
                    nc.sync.dma_start(wk[c][:], d["d_wk"][r0:r0 + P, :])
                    nc.sync.dma_start(wv[c][:], d["d_wv"][r0:r0 + P, :])
                for c in range(NC3):
                    r0 = l * C + c * P
                    nc.sync.dma_start(wo[c][:], d["d_wo"][r0:r0 + P, :])
                    nc.sync.dma_start(w1[c][:], d["d_w1"][r0:r0 + P, :])
                    nc.sync.dma_start(w2[c][:], d["d_w2"][r0:r0 + P, :])
                bqkt = [wpool.tile([P, 2], F32, name=f"bqkt{c}",
                                   tag=f"bqkt{c}") for c in range(NC3)]
                b1t = [wpool.tile([P, 1], F32, name=f"b1t{c}",
                                  tag=f"b1t{c}") for c in range(NC3)]
                for c in range(NC3):
                    r0 = l * C + c * P
                    nc.sync.dma_start(bqkt[c][:], d["d_bqkt"][r0:r0 + P, :])
                    nc.sync.dma_start(b1t[c][:], d["d_b1t"][r0:r0 + P, :])
                bv = wpool.tile([1, C], F32R, name="bv", tag="bv")
                bo = wpool.tile([1, C], F32R, name="bo", tag="bo")
                b2 = wpool.tile([1, C], F32R, name="b2", tag="b2")
                nc.sync.dma_start(bv[:], d["d_bv"][l:l + 1, :])
                nc.sync.dma_start(bo[:], d["d_bo"][l:l + 1, :])
                nc.sync.dma_start(b2[:], d["d_b2"][l:l + 1, :])

                with ExitStack() as actx:
                    attpool = actx.enter_context(
                        tc.tile_pool(name=f"attpool{l}", bufs=3))
                    attT = [attpool.tile([P, N], F32R, name="attT", tag="attT")
                            for _ in range(NC3)]
                    with ExitStack() as qctx:
                        atp = qctx.enter_context(
                            tc.tile_pool(name=f"atp{l}", bufs=1))
                        _mark(nc, f"L{l}.ln1")
                        aT = _layernorm_transposed(
                            nc, tc, h, eps, ident, atp, spool, f"a{l}", ev,
                            "aT")
                        _mark(nc, f"L{l}.v")

                        vpool = qctx.enter_context(
                            tc.tile_pool(name=f"vpool{l}", bufs=NT))
                        psc = qctx.enter_context(tc.tile_pool(
                            name=f"psc{l}", bufs=2, space="PSUM"))
                        psa = qctx.enter_context(tc.tile_pool(
                            name=f"psa{l}", bufs=2, space="PSUM"))
                        v = []
                        for i in range(NT):
                            ps = psc.tile([P, C], F32, name="psc", tag="psc")
                            nc.tensor.matmul(ps[:], ones[:, :P], bv[:],
                                             start=True, stop=False)
                            for c in range(NC3):
                                nc.tensor.matmul(
                                    ps[:], aT[c][:, i * P:(i + 1) * P],
                                    wv[c][:], start=False,
                                    stop=(c == NC3 - 1))
                            v_i = vpool.tile([P, C], F32R, name="v", tag="v")
                            ev.copy(v_i[:], ps[:])
                            v.append(v_i)

                        qkpool = qctx.enter_context(
                            tc.tile_pool(name=f"qkpool{l}", bufs=2))
                        ppool = qctx.enter_context(
                            tc.tile_pool(name=f"ppool{l}", bufs=4))
                        vspool = qctx.enter_context(
                            tc.tile_pool(name=f"vspool{l}", bufs=6))
                        _mark(nc, f"L{l}.attn")
                        def build_qk(m):
                            qT_m = qkpool.tile([P, N], F32R, name="qT",
                                               tag="qT")
                            kT_m = qkpool.tile([P, N], F32R, name="kT",
                                               tag="kT")
                            for dst, wmat, bcol in (
                                    (qT_m, wq, bqkt[m][:, 0:1]),
                                    (kT_m, wk, bqkt[m][:, 1:2])):
                                for t4 in range(N // 512):
                                    ps = psc.tile([P, 512], F32, name="psc",
                                                  tag="psc")
                                    for c in range(NC3):
                                        nc.tensor.matmul(
                                            ps[:],
                                            wmat[c][:, m * P:(m + 1) * P],
                                            aT[c][:, t4 * 512:(t4 + 1) * 512],
                                            start=(c == 0),
                                            stop=(c == NC3 - 1))
                                    nc.vector.scalar_tensor_tensor(
                                        dst[:, t4 * 512:(t4 + 1) * 512],
                                        ps[:], bcol,
                                        dst[:, t4 * 512:(t4 + 1) * 512],
                                        op0=ALU.add, op1=ALU.bypass)
                            return qT_m, kT_m

                        qk_next = build_qk(0)
                        for m in range(NC3):
                            qT_m, kT_m = qk_next
                            _attention_bm(nc, tc, l, 0, m, qT_m, kT_m, v,
                                          attT, ones, trib, identb,
                                          ppool, vspool, spool, psc, psa,
                                          ev)
                            if m + 1 < NC3:
                                qk_next = build_qk(m + 1)
                            _attention_bm(nc, tc, l, 1, m, qT_m, kT_m, v,
                                          attT, ones, trib, identb,
                                          ppool, vspool, spool, psc, psa,
                                          ev)

                    _mark(nc, f"L{l}.proj")
                    # --- proj: h += attT.T @ Wo + bo ---
                    pso_cm = tc.tile_pool(name=f"pso{l}", bufs=2,
                                          space="PSUM")
                    pso = pso_cm.__enter__()
                    for i in range(NT):
                        ps = pso.tile([P, C], F32, name="pmm", tag="pmm")
                        nc.tensor.matmul(ps[:], ones[:, :P], bo[:],
                                         start=True, stop=False)
                        for c in range(NC3):
                            nc.tensor.matmul(
                                ps[:], attT[c][:, i * P:(i + 1) * P],
                                wo[c][:], start=False, stop=(c == NC3 - 1))
                        nc.vector.tensor_add(h[i][:], h[i][:], ps[:])
                    pso_cm.__exit__(None, None, None)

                # --- LN2 + MLP ---
                _mark(nc, f"L{l}.mlp")
                wlm = blm = None
                if l == NL - 1 and not has_blm:
                    # prefetch the LM-head weights under the last MLP
                    lmpool = lctx.enter_context(
                        tc.tile_pool(name="lmpool", bufs=1))
                    wlm = [lmpool.tile([P, VSH], F32R, name=f"wlm{c}",
                                       tag=f"wlm{c}") for c in range(NC3)]
                    for c in range(NC3):
                        nc.sync.dma_start(wlm[c][:],
                                          d["d_wlm"][c * P:(c + 1) * P, :])
                with ExitStack() as mctx:
                    atp2 = mctx.enter_context(
                        tc.tile_pool(name=f"atp2{l}", bufs=1))
                    psm = mctx.enter_context(tc.tile_pool(
                        name=f"psm{l}", bufs=4, space="PSUM"))
                    m1pool = mctx.enter_context(
                        tc.tile_pool(name=f"m1pool{l}", bufs=3))
                    a2T = _layernorm_transposed(
                        nc, tc, h, eps, ident, atp2, spool, f"b{l}", ev, "aT")
                    m1T = [m1pool.tile([P, N], F32R, name="m1T", tag="m1T")
                           for _ in range(NC3)]
                    for cm in range(NC3):
                        for t4 in range(N // 512):
                            ps = psm.tile([P, 512], F32, name="pmm", tag="pmm")
                            for c in range(NC3):
                                nc.tensor.matmul(
                                    ps[:], w1[c][:, cm * P:(cm + 1) * P],
                                    a2T[c][:, t4 * 512:(t4 + 1) * 512],
                                    start=(c == 0), stop=(c == NC3 - 1))
                            nc.scalar.activation(
                                m1T[cm][:, t4 * 512:(t4 + 1) * 512],
                                ps[:], AF.Relu, bias=b1t[cm][:, 0:1])
                    for i in range(NT):
                        ps = psm.tile([P, C], F32, name="pmm", tag="pmm")
                        nc.tensor.matmul(ps[:], ones[:, :P], b2[:],
                                         start=True, stop=False)
                        for c in range(NC3):
                            nc.tensor.matmul(
                                ps[:], m1T[c][:, i * P:(i + 1) * P],
                                w2[c][:], start=False, stop=(c == NC3 - 1))
                        nc.vector.tensor_add(h[i][:], h[i][:], ps[:])

                if l == NL - 1 and not has_blm:
                    _lm_head(nc, tc, d, h, ident, ones, ev, has_blm,
                             wlm, blm)

        if has_blm:
            with tc.tile_pool(name="lmpool", bufs=1) as lmpool:
                wlm = [lmpool.tile([P, VSH], F32R, name=f"wlm{c}",
                                   tag=f"wlm{c}") for c in range(NC3)]
                for c in range(NC3):
                    nc.sync.dma_start(wlm[c][:],
                                      d["d_wlm"][c * P:(c + 1) * P, :])
                blm = lmpool.tile([1, VSH], F32R, name="blm", tag="blm")
                nc.sync.dma_start(blm[:], d["d_blm"][:])
                _lm_head(nc, tc, d, h, ident, ones, ev, has_blm, wlm, blm)

def _lm_head(nc, tc, d, h, ident, ones, ev, has_blm, wlm, blm):
    from contextlib import ExitStack
    _mark(nc, "lmhead")
    # ------------- LM head -------------
    with ExitStack() as lmctx:
        htpool = lmctx.enter_context(tc.tile_pool(name="htpool", bufs=1))
        opool = lmctx.enter_context(tc.tile_pool(
            name="opool", bufs=1 if has_blm else 2))
        pslm = lmctx.enter_context(
            tc.tile_pool(name="pslm", bufs=6, space="PSUM"))
        hTw = htpool.tile([P, NC3 * N], F32R, name="hT",
                          tag="hT")
        hT = [bass.AP(tensor=hTw.tensor,
                      offset=hTw.offset + c * N,
                      ap=[hTw.ap[0], [1, N]])
              for c in range(NC3)]
        for i in range(NT):
            pt = pslm.tile([P, C], F32, name="plm", tag="plm")
            for c in range(NC3):
                nc.tensor.transpose(pt[:, c * P:(c + 1) * P],
                                    h[i][:, c * P:(c + 1) * P],
                                    ident[:])
            out_ap = bass.AP(tensor=hTw.tensor,
                             offset=hTw.offset + i * P,
                             ap=[hTw.ap[0], [N, NC3], [1, P]])
            ev.copy(out_ap, pt[:])

        nvb = (VSH + 511) // 512  # 13 vocab banks (last is 256 wide)
        halves = [(0, list(range(0, 6)), 3072),
                  (3072, list(range(6, nvb)), VSH - 3072)]
        for i in range(NT):
            for base, vgs_all, wcols in halves:
                ost = opool.tile([P, 3328], F32, name="ostage",
                                 tag="ostage")
                for g0 in range(0, len(vgs_all), 4):
                    vgs = vgs_all[g0:g0 + 4]
                    pss = {}
                    for vg in vgs:
                        nw = min(512, VSH - vg * 512)
                        pss[vg] = pslm.tile([P, 512], F32, name="plm",
                                           tag="plm")
                        if has_blm:
                            nc.tensor.matmul(
                                pss[vg][:, :nw], ones[:, :P],
                                blm[:, vg * 512:vg * 512 + nw],
                                start=True, stop=False)
                    for c in range(NC3):
                        for vg in vgs:
                            nw = min(512, VSH - vg * 512)
                            nc.tensor.matmul(
                                pss[vg][:, :nw],
                                hT[c][:, i * P:(i + 1) * P],
                                wlm[c][:, vg * 512:vg * 512 + nw],
                                start=(c == 0 and not has_blm),
                                stop=(c == NC3 - 1))
                    for vg in vgs:
                        nw = min(512, VSH - vg * 512)
                        ev.copy(ost[:, vg * 512 - base:vg * 512 - base + nw],
                                pss[vg][:, :nw])
                nc.sync.dma_start(
                    d["d_out"][i * P:(i + 1) * P, base:base + wcols],
                    ost[:, :wcols])




def _layernorm_transposed(nc, tc, h, eps, ident, atpool, spool, label,
                          ev, at_tag):
    """LN (affine folded into weights host-side) + transpose -> 3 chunks
    (128, 2048) fp32r."""
    mv = spool.tile([P, 2 * NT], F32, name="mv", tag="mv")
    for i in range(NT):
        st = spool.tile([P, 6], F32, name="st", tag="st")
        nc.vector.bn_stats(st[:], h[i][:])
        nc.vector.bn_aggr(mv[:, 2 * i:2 * i + 2], st[:])
    std = spool.tile([P, NT], F32, name="std", tag="std")
    rstd = spool.tile([P, NT], F32, name="rstd", tag="rstd")
    nmr = spool.tile([P, NT], F32, name="nmr", tag="nmr")
    for g in range(0, NT, 4):
        # finalize stats in groups of 4 chunks so downstream work on early
        # chunks does not wait for the last bn_stats (layer-0 head latency)
        nc.scalar.activation(std[:, g:g + 4], mv[:, 2 * g + 1:2 * g + 8:2],
                             AF.Sqrt, bias=eps[:, :1])
        nc.vector.reciprocal(rstd[:, g:g + 4], std[:, g:g + 4])
        nc.vector.scalar_tensor_tensor(nmr[:, g:g + 4],
                                       mv[:, 2 * g:2 * g + 8:2], -1.0,
                                       rstd[:, g:g + 4],
                                       op0=ALU.mult, op1=ALU.mult)
    aTw = atpool.tile([P, NC3 * N], F32R, name=at_tag, tag=at_tag)
    aT = [bass.AP(tensor=aTw.tensor, offset=aTw.offset + c * N,
                  ap=[aTw.ap[0], [1, N]]) for c in range(NC3)]
    with tc.tile_pool(name=f"apool{label}", bufs=3) as apool, \
         tc.tile_pool(name=f"pstr{label}", bufs=3, space="PSUM") as pstr:
        for i in range(NT):
            a_i = apool.tile([P, C], F32, name="a", tag="a")
            nc.vector.tensor_scalar(a_i[:], h[i][:], rstd[:, i:i + 1],
                                    nmr[:, i:i + 1], op0=ALU.mult,
                                    op1=ALU.add)
            pt = pstr.tile([P, C], F32, name="ptr", tag="ptr")
            for c in range(NC3):
                nc.tensor.transpose(pt[:, c * P:(c + 1) * P],
                                    a_i[:, c * P:(c + 1) * P], ident[:])
            out_ap = bass.AP(tensor=aTw.tensor,
                             offset=aTw.offset + i * P,
                             ap=[aTw.ap[0], [N, NC3], [1, P]])
            nc.scalar.copy(out_ap, pt[:])
    return aT


def _attention_bm(nc, tc, l, b, m, qT_m, kT_m, v, attT, ones, trib,
                  identb, ppool, vspool, spool, psc, psa, ev):
    """Scores + query-axis softmax + p@v for batch b, heads (2m, 2m+1).

    Scores for one (head, key-chunk) land in a (128, 1024) two-bank PSUM
    tile so a single Exp (with fused row-sum accum_out) covers the whole
    valid range [128*kc : 1024). The softmax denominator is folded into v
    rows (per-partition scale). att accumulates in (64, 1024) PSUM tiles
    with the two heads in separate banks (free halves)."""
    d0 = spool.tile([P, 16], F32, name="d0", tag="d0")
    dinv = spool.tile([P, 16], F32, name="dinv", tag="dinv")

    att_ps = {tb: psa.tile([64, 1024], F32, name="patt", tag="patt")
              for tb in range(TB)}
    pending = []

    for kc in range(KC):
        p_kc = ppool.tile([P, 2 * T], F32R, name="p", tag="p")
        ktok = b * T + kc * P
        lo_kc = 128 * kc
        for hh in range(2):
            pp = psc.tile([P, T], F32, name="psc", tag="psc")
            diag_tb = kc // 4
            dcol = 128 * kc
            nc.tensor.matmul(pp[:, dcol:dcol + P], identb[:], trib[:],
                             start=True, stop=False)
            for tb in range(TB):
                lo = _valid_lo(kc, tb)
                if lo is None:
                    continue
                lo_mm = min(lo, 256)  # keep fp32r moving dim >= 256
                nc.tensor.matmul(
                    pp[:, tb * 512 + lo_mm:(tb + 1) * 512],
                    kT_m[64 * hh:64 * hh + 64, ktok:ktok + P],
                    qT_m[64 * hh:64 * hh + 64,
                         b * T + tb * 512 + lo_mm:b * T + (tb + 1) * 512],
                    start=(tb != diag_tb), stop=(tb == TB - 1))
            nc.scalar.activation(
                p_kc[:, hh * T + lo_kc:(hh + 1) * T],
                pp[:, lo_kc:T], AF.Exp,
                accum_out=d0[:, 8 * hh + kc:8 * hh + kc + 1])

        # 1/denominator for both heads (cols kc, 8+kc), then fold into v
        nc.vector.reciprocal(dinv[:, kc::8], d0[:, kc::8])
        vs = vspool.tile([P, P], F32R, name="vs", tag="vs")
        it = (b * T + kc * P) // P
        for hh in range(2):
            vslice = v[it][:, m * P + 64 * hh:m * P + 64 * hh + 64]
            nc.vector.scalar_tensor_tensor(
                vs[:, 64 * hh:64 * hh + 64], vslice,
                dinv[:, 8 * hh + kc:8 * hh + kc + 1], vslice,
                op0=ALU.mult, op1=ALU.bypass)
        pending.append((kc, p_kc, vs))
        if len(pending) > 1:
            _emit_att(nc, attT, att_ps, m, b, *pending.pop(0))

    while pending:
        _emit_att(nc, attT, att_ps, m, b, *pending.pop(0))
    for hh in range(2):
        nc.vector.tensor_copy(
            attT[m][64 * hh:64 * hh + 64, b * T + 512:b * T + 1024],
            att_ps[1][:, hh * 512:(hh + 1) * 512])


# ---------------------------------------------------------------------------
# host side
# ---------------------------------------------------------------------------

def _prep_inputs(inputs):
    f32 = np.float32
    tok_emb = np.asarray(inputs["tok_emb"], f32)
    pos_emb = np.asarray(inputs["pos_emb"], f32)
    x = np.asarray(inputs["x"]).astype(np.int32).reshape(N, 1)

    def fold_qkv(W, bias, g, b_ln, extra=1.0):
        # W: (NL, H, C, HS) -> (NL*C, H*HS), rows scaled by g, * extra
        Wf = np.transpose(np.asarray(W, f32), (0, 2, 1, 3)).reshape(NL, C, C)
        bf = (np.asarray(bias, f32).reshape(NL, C)
              + np.einsum("lc,lcd->ld", np.asarray(b_ln, f32), Wf))
        Wg = Wf * np.asarray(g, f32)[:, :, None]
        return (Wg * extra).reshape(NL * C, C), (bf * extra)

    g1, b1n = inputs["ln1_g"], inputs["ln1_b"]
    g2, b2n = inputs["ln2_g"], inputs["ln2_b"]
    wq, bq = fold_qkv(inputs["Wq"], inputs["bq"], g1, b1n)
    wk, bk = fold_qkv(inputs["Wk"], inputs["bk"], g1, b1n, extra=HS ** -0.5)
    wv, bv = fold_qkv(inputs["Wv"], inputs["bv"], g1, b1n)

    W1 = np.asarray(inputs["W1"], f32)
    w1 = (W1 * np.asarray(g2, f32)[:, :, None])
    b1f = (np.asarray(inputs["b1"], f32)
           + np.einsum("lc,lcd->ld", np.asarray(b2n, f32), W1))

    tri = np.zeros((P, P), f32)
    tri[np.tril_indices(P, -1)] = NEG  # tri[k, t] = NEG where t < k
    import ml_dtypes
    trib = tri.astype(ml_dtypes.bfloat16)
    identb = np.eye(P, dtype=ml_dtypes.bfloat16)

    wlm_pad = np.zeros((C, VPAD), f32)
    wlm_pad[:, :V] = np.asarray(inputs["Wlm"], f32)
    blm_pad = np.zeros((1, VPAD), f32)
    blm_pad[0, :V] = np.asarray(inputs["blm"], f32)
    has_blm = bool(np.any(blm_pad))

    common = {
        "idx": x,
        "tok_emb": tok_emb,
        "pos": np.tile(pos_emb, (B, 1)),
        "wq": wq, "wk": wk, "wv": wv,
        "wo": np.asarray(inputs["Wo"], f32).reshape(NL * C, C),
        "w1": w1.reshape(NL * C, C),
        "w2": np.asarray(inputs["W2"], f32).reshape(NL * C, C),
        "bq": bq, "bk": bk, "bv": bv,
        "bo": np.asarray(inputs["bo"], f32),
        "b1": b1f,
        "b2": np.asarray(inputs["b2"], f32),
        "ones": np.ones((1, 512), f32),
        "ident": np.eye(P, dtype=f32),
        "identb": identb,
        "trib": trib,
        "bqkt": np.stack([bq.reshape(-1), bk.reshape(-1)], axis=1),
        "b1t": b1f.reshape(-1, 1),
    }
    in_maps = []
    for j in range(NCORE):
        im = dict(common)
        im["wlm"] = np.ascontiguousarray(wlm_pad[:, j * VSH:(j + 1) * VSH])
        if has_blm:
            im["blm"] = np.ascontiguousarray(blm_pad[:, j * VSH:(j + 1) * VSH])
        in_maps.append(im)
    return in_maps, has_blm


def kernel(**inputs):
    in_maps, has_blm = _prep_inputs(inputs)
    key = ("nc", has_blm)
    if key not in _CACHE:
        _CACHE[key] = _build(has_blm)
    nc = _CACHE[key]
    res = bass_utils.run_bass_kernel_spmd(nc, in_maps,
                                          core_ids=list(range(NCORE)))
    logits = np.concatenate([r["logits"] for r in res.results], axis=1)
    return logits[:, :V].reshape(B, T, V)


if __name__ == "__main__":
    pass

def _emit_att(nc, attT, att_ps, m, b, kc, p_kc, vs):
    for hh in range(2):
        for tb in range(TB):
            lo = _valid_lo(kc, tb)
            if lo is None:
                continue
            last = (kc == (3 if tb == 0 else KC - 1))
            nc.tensor.matmul(
                att_ps[tb][:, hh * 512 + lo:(hh + 1) * 512],
                vs[:, 64 * hh:64 * hh + 64],
                p_kc[:, hh * T + tb * 512 + lo:hh * T + (tb + 1) * 512],
                start=(kc == 0), stop=last, skip_group_check=True)
    if kc == 3:
        for hh in range(2):
            nc.vector.tensor_copy(
                attT[m][64 * hh:64 * hh + 64, b * T:b * T + 512],
                att_ps[0][:, hh * 512:(hh + 1) * 512])



# revision 21
# speedup vs baseline: 1.5467x; 1.5467x over previous
"""Bass/Trainium2 kernel for nn_BigramLanguageModel (v3).

Sharding (8 NeuronCores, single SPMD launch, no collectives):
  - core j: batch b = j//4, vocab quarter q = j%4. Each core runs the
    3-layer transformer on its 1024-token batch (2-way data parallel)
    and computes logits[:, 12800*q : 12800*(q+1)] (4-way tensor
    parallel over the padded 51200 vocab). Host concatenates.
  - All matmul operands are bf16 (fp32 PSUM accumulation, fp32 residual
    stream h in SBUF). Logits leave the core as bf16 (halves the output
    DMA) and are upcast on host. rel-err budget 2e-2 >> bf16 ~2e-3.
  - LayerNorm affine is folded into the following projections
    host-side; 1/sqrt(HS) into Wk; q/k biases into the PSUM
    evacuations; b1 into the Relu evacuation; v/o/mlp2 biases ride as
    rank-1 PSUM-preload matmuls.
  - All 128x128 block transposes (LN outputs, LM-head h) run on the
    DMA engines (InstDmaTransposeAnt, 14ns/xbar-tile) instead of the
    PE array; the normalized activations are written token-major so
    one DMA transposes 2 tiles per call.
  - Softmax over the *query* axis in transposed score layout wT[k, t]:
    one Exp per (head, key-chunk); the denominator is a pass-through
    scalar_tensor_tensor on DVE (4x bf16 mode) with fused accum_out;
    1/denom folds into v rows via an ALU-divide on the Pool engine
    (gpsimd: SBUF-only work — it has no PSUM port).
  - Causal mask via bf16 (-80)-triangle PSUM-preload matmul.
  - Per-layer weights arrive as ONE packed DMA; LM-head weights
    prefetch at the start of the last layer; the LM head is fused into
    the last MLP tile loop so its GEMMs overlap the transformer tail.
"""

import sys

sys.path.insert(0, "/opt/trn_rl_repo")

import numpy as np

import concourse.bass as bass
import concourse.mybir as mybir
import concourse.tile as tile
from concourse import bacc
from concourse import bass_utils

F32 = mybir.dt.float32
BF16 = mybir.dt.bfloat16
I32 = mybir.dt.int32
AF = mybir.ActivationFunctionType
ALU = mybir.AluOpType

V, C, T, H, HS, NL, B = 50257, 384, 1024, 6, 64, 3, 2
P = 128
N = T                      # 1024 tokens per core (one batch)
NT = N // P                # 8 token chunks
NC3 = C // P               # 3 channel chunks
NCORE = 8
NQ = 4                     # vocab quarters
VPAD = 51200               # padded vocab (4 * 12800)
VSH = VPAD // NQ           # 12800 vocab columns per core
KC = T // P                # 8 key chunks
TB = T // 512              # 2 query blocks of 512
NEG = -80.0                # mask bias (exp(-80) ~ 1.8e-35)
NW = 18                    # packed weight tiles per layer (6 mats x 3 chunks)

_CACHE: dict = {}
PHASES: list = []


def _mark(nc, label):
    PHASES.append((label, int(nc.next_id())))


class _EvacSplit:
    """Round-robin PSUM->SBUF evacuation copies over DVE / ACT.
    (gpsimd has no PSUM port, so Pool is not in this rotation.)"""

    def __init__(self, nc):
        self.nc = nc
        self.i = 0

    def copy(self, out, in_):
        self.i += 1
        if self.i % 2 == 0:
            self.nc.vector.tensor_copy(out, in_)
        else:
            self.nc.scalar.copy(out, in_)


def _build(has_blm: bool):
    nc = bacc.Bacc("TRN2", target_bir_lowering=False, debug=False)

    d_idx = nc.dram_tensor("idx", [N, 1], I32, kind="ExternalInput").ap()
    d_tok = nc.dram_tensor("tok_emb", [V, C], BF16, kind="ExternalInput").ap()
    d_pos = nc.dram_tensor("pos", [N, C], BF16, kind="ExternalInput").ap()
    # packed per-layer weights: rows (l*NW + k)*P + p, k = mat*3 + chunk
    d_wall = nc.dram_tensor("wall", [NL * NW * P, C], BF16,
                            kind="ExternalInput").ap()
    # packed per-partition bias columns: [bq | bk | b1] per layer
    d_bcol = nc.dram_tensor("bcol", [NL * C, 3], F32,
                            kind="ExternalInput").ap()
    # packed bias rows [bv ; bo ; b2] per layer
    d_brow = nc.dram_tensor("brow", [NL * 3, C], BF16,
                            kind="ExternalInput").ap()
    d_ones = nc.dram_tensor("ones", [1, 512], BF16, kind="ExternalInput").ap()
    d_identb = nc.dram_tensor("identb", [P, P], BF16,
                              kind="ExternalInput").ap()
    d_trib = nc.dram_tensor("trib", [P, P], BF16, kind="ExternalInput").ap()
    d_wlm = nc.dram_tensor("wlm", [C, VSH], BF16, kind="ExternalInput").ap()
    if has_blm:
        d_blm = nc.dram_tensor("blm", [1, VSH], BF16,
                               kind="ExternalInput").ap()
    d_out = nc.dram_tensor("logits", [N, VSH], BF16,
                           kind="ExternalOutput").ap()

    with tile.TileContext(nc) as tc:
        _emit(nc, tc, locals(), has_blm)
    nc.compile()
    return nc


def _emit(nc, tc, d, has_blm):
    from contextlib import ExitStack

    with ExitStack() as ctx:
        hpool = ctx.enter_context(tc.tile_pool(name="hpool", bufs=NT))
        pers = ctx.enter_context(tc.tile_pool(name="pers", bufs=1))
        spool = ctx.enter_context(tc.tile_pool(name="spool", bufs=8))

        # ------------- embedding gather (DMAs issued first) -------------
        _mark(nc, "embed")
        h = []  # 8 residual-stream tiles (128, 384) fp32
        idx_all = pers.tile([P, NT], I32, name="idx", tag="idx")
        nc.sync.dma_start(
            idx_all[:],
            bass.AP(tensor=d["d_idx"].tensor, offset=d["d_idx"].offset,
                    ap=[[1, P], [P, NT]]))
        posw = pers.tile([P, NT * C], BF16, name="pos", tag="pos")
        nc.sync.dma_start(
            posw[:],
            bass.AP(tensor=d["d_pos"].tensor, offset=d["d_pos"].offset,
                    ap=[[C, P], [P * C, NT], [1, C]]))

        # ------------- constants -------------
        ones = pers.tile([1, 512], BF16, name="ones", tag="ones")
        identb = pers.tile([P, P], BF16, name="identb", tag="identb")
        trib = pers.tile([P, P], BF16, name="trib", tag="trib")
        eps = pers.tile([P, 1], F32, name="eps", tag="eps")
        nc.sync.dma_start(ones[:], d["d_ones"][:])
        nc.sync.dma_start(identb[:], d["d_identb"][:])
        nc.sync.dma_start(trib[:], d["d_trib"][:])
        nc.vector.memset(eps[:], 1e-5)

        with tc.tile_pool(name="epool", bufs=3) as epool:
            for i in range(NT):
                emb = epool.tile([P, C], BF16, name="emb", tag="emb")
                nc.gpsimd.indirect_dma_start(
                    out=emb[:], out_offset=None, in_=d["d_tok"][:],
                    in_offset=bass.IndirectOffsetOnAxis(
                        ap=idx_all[:, i:i + 1], axis=0),
                )
                h_i = hpool.tile([P, C], F32, name="h", tag="h")
                nc.vector.tensor_add(h_i[:], emb[:],
                                     posw[:, i * C:(i + 1) * C])
                h.append(h_i)

        ev = _EvacSplit(nc)

        # LM-head weights: persistent tiles, prefetched in small chunks
        # from layer 1 onward so the transfers never head-of-line block
        # the LN transpose DMAs on the (serialized) DMA engines.
        lmpool = ctx.enter_context(tc.tile_pool(name="lmpool", bufs=1))
        wlm = [lmpool.tile([P, VSH], BF16, name=f"wlm{c}", tag=f"wlm{c}")
               for c in range(NC3)]
        blm = None
        if has_blm:
            blm = lmpool.tile([1, VSH], BF16, name="blm", tag="blm")

        # ------------- layers -------------
        for l in range(NL):
            with ExitStack() as lctx:
                wpool = lctx.enter_context(
                    tc.tile_pool(name=f"wpool{l}", bufs=1))
                wall = wpool.tile([P, NW * C], BF16, name="wall", tag="wall")
                r0 = l * NW * P
                for half in range(2):
                    nc.sync.dma_start(
                        wall[:, half * 9 * C:(half + 1) * 9 * C],
                        bass.AP(tensor=d["d_wall"].tensor,
                                offset=(d["d_wall"].offset
                                        + (r0 + half * 9 * P) * C),
                                ap=[[C, P], [P * C, 9], [1, C]]))

                def wslice(mat, c):
                    k = mat * 3 + c
                    return wall[:, k * C:(k + 1) * C]

                wq = [wslice(0, c) for c in range(NC3)]
                wk = [wslice(1, c) for c in range(NC3)]
                wv = [wslice(2, c) for c in range(NC3)]
                wo = [wslice(3, c) for c in range(NC3)]
                w1 = [wslice(4, c) for c in range(NC3)]
                w2 = [wslice(5, c) for c in range(NC3)]

                bcol = wpool.tile([P, NC3 * 3], F32, name="bcol", tag="bcol")
                nc.sync.dma_start(
                    bcol[:],
                    bass.AP(tensor=d["d_bcol"].tensor,
                            offset=d["d_bcol"].offset + l * C * 3,
                            ap=[[3, P], [P * 3, NC3], [1, 3]]))
                bqkt = [bcol[:, 3 * c:3 * c + 2] for c in range(NC3)]
                b1t = [bcol[:, 3 * c + 2:3 * c + 3] for c in range(NC3)]
                brow = wpool.tile([1, 3 * C], BF16, name="brow", tag="brow")
                nc.sync.dma_start(
                    brow[:],
                    bass.AP(tensor=d["d_brow"].tensor,
                            offset=d["d_brow"].offset + l * 3 * C,
                            ap=[[3 * C, 1], [1, 3 * C]]))
                bv = brow[:, 0:C]
                bo = brow[:, C:2 * C]
                b2 = brow[:, 2 * C:3 * C]

                if l == 1:
                    # prefetch LM-head weights in 12 small chunks
                    for c in range(NC3):
                        for q4 in range(4):
                            nc.sync.dma_start(
                                wlm[c][:, q4 * 3200:(q4 + 1) * 3200],
                                d["d_wlm"][c * P:(c + 1) * P,
                                           q4 * 3200:(q4 + 1) * 3200])
                    if has_blm:
                        nc.sync.dma_start(blm[:], d["d_blm"][:])

                with ExitStack() as actx:
                    attpool = actx.enter_context(
                        tc.tile_pool(name=f"attpool{l}", bufs=1))
                    attT = [attpool.tile([P, N], BF16, name=f"attT{c}",
                                         tag=f"attT{c}")
                            for c in range(NC3)]
                    with ExitStack() as qctx:
                        atp = qctx.enter_context(
                            tc.tile_pool(name=f"atp{l}", bufs=1))
                        _mark(nc, f"L{l}.ln1")
                        at = _layernorm_transposed(
                            nc, tc, h, eps, atp, spool, f"a{l}")
                        _mark(nc, f"L{l}.v")

                        vpool = qctx.enter_context(
                            tc.tile_pool(name=f"vpool{l}", bufs=NT))
                        psc = qctx.enter_context(tc.tile_pool(
                            name=f"psc{l}", bufs=2, space="PSUM"))
                        psa = qctx.enter_context(tc.tile_pool(
                            name=f"psa{l}", bufs=2, space="PSUM"))
                        v = []
                        for i in range(NT):
                            ps = psc.tile([P, C], F32, name="psc", tag="psc")
                            nc.tensor.matmul(ps[:], ones[:, :P], bv,
                                             start=True, stop=False)
                            for c in range(NC3):
                                nc.tensor.matmul(
                                    ps[:], at(c, i), wv[c], start=False,
                                    stop=(c == NC3 - 1))
                            v_i = vpool.tile([P, C], BF16, name="v", tag="v")
                            ev.copy(v_i[:], ps[:])
                            v.append(v_i)

                        qkpool = qctx.enter_context(
                            tc.tile_pool(name=f"qkpool{l}", bufs=2))
                        ppool = qctx.enter_context(
                            tc.tile_pool(name=f"ppool{l}", bufs=4))
                        vspool = qctx.enter_context(
                            tc.tile_pool(name=f"vspool{l}", bufs=6))
                        _mark(nc, f"L{l}.attn")

                        def build_qk(m):
                            qT_m = qkpool.tile([P, N], BF16, name="qT",
                                               tag="qT")
                            kT_m = qkpool.tile([P, N], BF16, name="kT",
                                               tag="kT")
                            for dst, wmat, bc in (
                                    (qT_m, wq, bqkt[m][:, 0:1]),
                                    (kT_m, wk, bqkt[m][:, 1:2])):
                                for t4 in range(N // 512):
                                    ps = psc.tile([P, 512], F32, name="psc",
                                                  tag="psc")
                                    for c in range(NC3):
                                        nc.tensor.matmul(
                                            ps[:],
                                            wmat[c][:, m * P:(m + 1) * P],
                                            at(c, t4, blk=True),
                                            start=(c == 0),
                                            stop=(c == NC3 - 1))
                                    nc.vector.scalar_tensor_tensor(
                                        dst[:, t4 * 512:(t4 + 1) * 512],
                                        ps[:], bc,
                                        dst[:, t4 * 512:(t4 + 1) * 512],
                                        op0=ALU.add, op1=ALU.bypass)
                            return qT_m, kT_m

                        qk_next = build_qk(0)
                        for m in range(NC3):
                            qT_m, kT_m = qk_next
                            qk_next = None
                            _attention_m(
                                nc, l, m, qT_m, kT_m, v, attT, trib,
                                identb, ppool, vspool, spool, psc, psa, ev,
                                mid=(lambda mm=m: build_qk(mm + 1))
                                if m + 1 < NC3 else None)
                            if m + 1 < NC3:
                                qk_next = _attention_m.qk_built

                    _mark(nc, f"L{l}.proj")
                    with tc.tile_pool(name=f"pso{l}", bufs=2,
                                      space="PSUM") as pso:
                        for i in range(NT):
                            ps = pso.tile([P, C], F32, name="pmm", tag="pmm")
                            nc.tensor.matmul(ps[:], ones[:, :P], bo,
                                             start=True, stop=False)
                            for c in range(NC3):
                                nc.tensor.matmul(
                                    ps[:], attT[c][:, i * P:(i + 1) * P],
                                    wo[c], start=False, stop=(c == NC3 - 1))
                            nc.vector.tensor_add(h[i][:], h[i][:], ps[:])

                # --- LN2 + MLP (+ fused LM head on the last layer) ---
                _mark(nc, f"L{l}.mlp")
                with ExitStack() as mctx:
                    atp2 = mctx.enter_context(
                        tc.tile_pool(name=f"atp2{l}", bufs=1))
                    m1pool = mctx.enter_context(
                        tc.tile_pool(name=f"m1pool{l}", bufs=3))
                    a2t = _layernorm_transposed(
                        nc, tc, h, eps, atp2, spool, f"b{l}")
                    if l < NL - 1:
                        psm = mctx.enter_context(tc.tile_pool(
                            name=f"psm{l}", bufs=4, space="PSUM"))
                        ps_m1 = psm
                    else:
                        pstm = mctx.enter_context(tc.tile_pool(
                            name="pstm", bufs=2, space="PSUM"))
                        pslm = mctx.enter_context(tc.tile_pool(
                            name="pslm", bufs=6, space="PSUM"))
                        ps_m1 = pslm
                    m1T = [m1pool.tile([P, N], BF16, name="m1T", tag="m1T")
                           for _ in range(NC3)]
                    for cm in range(NC3):
                        for t4 in range(N // 512):
                            ps = ps_m1.tile([P, 512], F32, name="plm",
                                            tag="plm")
                            for c in range(NC3):
                                nc.tensor.matmul(
                                    ps[:], w1[c][:, cm * P:(cm + 1) * P],
                                    a2t(c, t4, blk=True),
                                    start=(c == 0), stop=(c == NC3 - 1))
                            nc.scalar.activation(
                                m1T[cm][:, t4 * 512:(t4 + 1) * 512],
                                ps[:], AF.Relu, bias=b1t[cm][:, 0:1])

                    if l < NL - 1:
                        for i in range(NT):
                            ps = psm.tile([P, C], F32, name="pmm", tag="pmm")
                            nc.tensor.matmul(ps[:], ones[:, :P], b2,
                                             start=True, stop=False)
                            for c in range(NC3):
                                nc.tensor.matmul(
                                    ps[:], m1T[c][:, i * P:(i + 1) * P],
                                    w2[c], start=False, stop=(c == NC3 - 1))
                            nc.vector.tensor_add(h[i][:], h[i][:], ps[:])
                    else:
                        _mark(nc, "lmhead")
                        hbpool = mctx.enter_context(
                            tc.tile_pool(name="hbpool", bufs=2))
                        opool = mctx.enter_context(
                            tc.tile_pool(name="opool", bufs=2))
                        for i in range(NT):
                            ps = pstm.tile([P, C], F32, name="pmm2",
                                           tag="pmm2")
                            nc.tensor.matmul(ps[:], ones[:, :P], b2,
                                             start=True, stop=False)
                            for c in range(NC3):
                                nc.tensor.matmul(
                                    ps[:], m1T[c][:, i * P:(i + 1) * P],
                                    w2[c], start=False, stop=(c == NC3 - 1))
                            hb = hbpool.tile([P, C], BF16, name="hb",
                                             tag="hb")
                            nc.vector.tensor_add(hb[:], h[i][:], ps[:])
                            # transpose hb -> hT (3 chunks) on the DMA xbar
                            hT = hbpool.tile([P, C], BF16, name="hT",
                                             tag="hT")
                            hT_ap = bass.AP(tensor=hT.tensor,
                                            offset=hT.offset,
                                            ap=[hT.ap[0], [P, NC3], [1, P]])
                            nc.scalar.dma_start_transpose(hT_ap, hb[:])
                            # 25 x 512-wide logits tiles for this token
                            # chunk, stationary operand outermost in groups
                            # of 3 (1 Ldweights per (group, c))
                            ost = opool.tile([P, VSH], BF16, name="ost",
                                             tag="ost")
                            nvg = VSH // 512
                            for g0 in range(0, nvg, 3):
                                vgs = range(g0, min(g0 + 3, nvg))
                                pvs = {}
                                for vg in vgs:
                                    pvs[vg] = pslm.tile([P, 512], F32,
                                                        name="plm",
                                                        tag="plm")
                                    if has_blm:
                                        nc.tensor.matmul(
                                            pvs[vg][:], ones[:, :P],
                                            blm[:, vg * 512:(vg + 1) * 512],
                                            start=True, stop=False)
                                for c in range(NC3):
                                    for vg in vgs:
                                        nc.tensor.matmul(
                                            pvs[vg][:],
                                            hT[:, c * P:(c + 1) * P],
                                            wlm[c][:,
                                                   vg * 512:(vg + 1) * 512],
                                            start=(c == 0 and not has_blm),
                                            stop=(c == NC3 - 1))
                                for vg in vgs:
                                    ev.copy(ost[:, vg * 512:(vg + 1) * 512],
                                            pvs[vg][:])
                            nc.sync.dma_start(
                                d["d_out"][i * P:(i + 1) * P, :], ost[:])


def _layernorm_transposed(nc, tc, h, eps, atpool, spool, label):
    """LN (affine folded into weights host-side); the normalized
    activations are written token-major into aW and transposed by the
    DMA xbar into aT (24 x 128x128 blocks at stride 128). Returns an
    accessor at(c, i) -> [128,128] chunk-c block of token tile i, or
    at(c, t4, blk=True) -> [128, 512] strided 4-block moving operand."""
    mv = spool.tile([P, 2 * NT], F32, name="mv", tag="mv")
    for i in range(NT):
        st = spool.tile([P, 6], F32, name="st", tag="st")
        nc.vector.bn_stats(st[:], h[i][:])
        nc.vector.bn_aggr(mv[:, 2 * i:2 * i + 2], st[:])
    std = spool.tile([P, NT], F32, name="std", tag="std")
    rstd = spool.tile([P, NT], F32, name="rstd", tag="rstd")
    nmr = spool.tile([P, NT], F32, name="nmr", tag="nmr")
    for g in range(0, NT, 4):
        nc.scalar.activation(std[:, g:g + 4], mv[:, 2 * g + 1:2 * g + 8:2],
                             AF.Sqrt, bias=eps[:, :1])
        nc.vector.reciprocal(rstd[:, g:g + 4], std[:, g:g + 4])
        nc.vector.scalar_tensor_tensor(nmr[:, g:g + 4],
                                       mv[:, 2 * g:2 * g + 8:2], -1.0,
                                       rstd[:, g:g + 4],
                                       op0=ALU.mult, op1=ALU.mult)
    aW = atpool.tile([P, NT * C], BF16, name=f"aW{label}", tag="aW")
    aTw = atpool.tile([P, NT * C], BF16, name=f"aT{label}", tag="aT")
    for i in range(NT):
        eng = nc.gpsimd if i % 2 == 0 else nc.vector
        eng.tensor_scalar(aW[:, i * C:(i + 1) * C], h[i][:],
                          rstd[:, i:i + 1], nmr[:, i:i + 1],
                          op0=ALU.mult, op1=ALU.add)
    for i2 in range(NT // 2):
        out_ap = bass.AP(tensor=aTw.tensor,
                         offset=aTw.offset + i2 * 2 * C,
                         ap=[aTw.ap[0], [P, 2 * NC3], [1, P]])
        nc.scalar.dma_start_transpose(out_ap, aW[:, i2 * 2 * C:
                                                 (i2 + 1) * 2 * C])

    def at(c, i, blk=False):
        if blk:  # 512 tokens: 4 tiles of 128 at stride C
            return bass.AP(tensor=aTw.tensor,
                           offset=aTw.offset + i * 4 * C + c * P,
                           ap=[aTw.ap[0], [C, 4], [1, P]])
        return bass.AP(tensor=aTw.tensor,
                       offset=aTw.offset + i * C + c * P,
                       ap=[aTw.ap[0], [1, P]])

    return at


def _attention_m(nc, l, m, qT_m, kT_m, v, attT, trib, identb,
                 ppool, vspool, spool, psc, psa, ev, mid=None):
    """Scores + query-axis softmax + p@v for heads (2m, 2m+1).

    Scores for one (head, key-chunk) land in a (128, 1024) two-bank PSUM
    tile; one Exp covers the causally-valid range [128*kc : 1024). The
    denominator comes from a pass-through STT on DVE (4x bf16) with
    fused accum_out; v rows are divided by it on the Pool engine. p@v
    accumulates in (64, 1024) PSUM tiles, two heads in separate free
    halves, pipelined 2 key-chunks behind the scores."""
    d0 = spool.tile([P, 16], F32, name="d0", tag="d0")
    dsc = spool.tile([P, T], BF16, name="dsc", tag="dsc")

    att_ps = {tb: psa.tile([64, 1024], F32, name="patt", tag="patt")
              for tb in range(TB)}
    pending = []

    for kc in range(KC):
        p_kc = ppool.tile([P, 2 * T], BF16, name="p", tag="p")
        lo_kc = 128 * kc
        w_kc = T - lo_kc
        diag_tb = kc // (512 // P)
        for hh in range(2):
            pp = psc.tile([P, T], F32, name="psc", tag="psc")
            nc.tensor.matmul(pp[:, lo_kc:lo_kc + P], identb[:], trib[:],
                             start=True, stop=False)
            for tb in range(TB):
                lo = 128 * kc - 512 * tb
                if lo >= 512:
                    continue
                lo = max(lo, 0)
                nc.tensor.matmul(
                    pp[:, tb * 512 + lo:(tb + 1) * 512],
                    kT_m[64 * hh:64 * hh + 64, lo_kc:lo_kc + P],
                    qT_m[64 * hh:64 * hh + 64,
                         tb * 512 + lo:(tb + 1) * 512],
                    start=(tb != diag_tb), stop=(tb == TB - 1))
            nc.scalar.activation(
                p_kc[:, hh * T + lo_kc:(hh + 1) * T],
                pp[:, lo_kc:T], AF.Exp)
            # denominator: pass-through tensor_scalar (4x bf16 on DVE)
            # with fused row-sum accum
            nc.vector.tensor_scalar(
                dsc[:, :w_kc], p_kc[:, hh * T + lo_kc:(hh + 1) * T],
                0.0, None, op0=ALU.add,
                accum_out=d0[:, 8 * hh + kc:8 * hh + kc + 1])

        vs = vspool.tile([P, P], BF16, name="vs", tag="vs")
        for hh in range(2):
            vslice = v[kc][:, m * P + 64 * hh:m * P + 64 * hh + 64]
            nc.gpsimd.scalar_tensor_tensor(
                vs[:, 64 * hh:64 * hh + 64], vslice,
                d0[:, 8 * hh + kc:8 * hh + kc + 1], vslice,
                op0=ALU.divide, op1=ALU.bypass)
        pending.append((kc, p_kc, vs))
        if len(pending) > 2:
            _emit_att(nc, attT, att_ps, m, *pending.pop(0))

    if mid is not None:
        _attention_m.qk_built = mid()
    while pending:
        _emit_att(nc, attT, att_ps, m, *pending.pop(0))
    for hh in range(2):
        if hh == 0:
            nc.vector.tensor_copy(
                attT[m][0:64, 512:1024], att_ps[1][:, 0:512])
        else:
            nc.scalar.copy(
                attT[m][64:128, 512:1024], att_ps[1][:, 512:1024])


def _emit_att(nc, attT, att_ps, m, kc, p_kc, vs):
    for tb in range(TB):
        lo = 128 * kc - 512 * tb
        if lo >= 512:
            continue
        lo = max(lo, 0)
        last = (kc == (3 if tb == 0 else KC - 1))
        for hh in range(2):
            nc.tensor.matmul(
                att_ps[tb][:, hh * 512 + lo:(hh + 1) * 512],
                vs[:, 64 * hh:64 * hh + 64],
                p_kc[:, hh * T + tb * 512 + lo:hh * T + (tb + 1) * 512],
                start=(kc == 0), stop=last, skip_group_check=True)
    if kc == 3:
        nc.vector.tensor_copy(attT[m][0:64, 0:512], att_ps[0][:, 0:512])
        nc.scalar.copy(attT[m][64:128, 0:512], att_ps[0][:, 512:1024])


# ---------------------------------------------------------------------------
# host side
# ---------------------------------------------------------------------------

def _prep_inputs(inputs):
    import ml_dtypes
    f32 = np.float32
    bf16 = ml_dtypes.bfloat16
    tok_emb = np.asarray(inputs["tok_emb"], f32)
    pos_emb = np.asarray(inputs["pos_emb"], f32)
    x = np.asarray(inputs["x"]).astype(np.int32)  # (B, T)

    def fold_qkv(W, bias, g, b_ln, extra=1.0):
        Wf = np.transpose(np.asarray(W, f32), (0, 2, 1, 3)).reshape(NL, C, C)
        bf = (np.asarray(bias, f32).reshape(NL, C)
              + np.einsum("lc,lcd->ld", np.asarray(b_ln, f32), Wf))
        Wg = Wf * np.asarray(g, f32)[:, :, None]
        return (Wg * extra), (bf * extra)

    g1, b1n = inputs["ln1_g"], inputs["ln1_b"]
    g2, b2n = inputs["ln2_g"], inputs["ln2_b"]
    wq, bq = fold_qkv(inputs["Wq"], inputs["bq"], g1, b1n)
    wk, bk = fold_qkv(inputs["Wk"], inputs["bk"], g1, b1n, extra=HS ** -0.5)
    wv, bv = fold_qkv(inputs["Wv"], inputs["bv"], g1, b1n)

    W1 = np.asarray(inputs["W1"], f32)
    w1 = W1 * np.asarray(g2, f32)[:, :, None]
    b1f = (np.asarray(inputs["b1"], f32)
           + np.einsum("lc,lcd->ld", np.asarray(b2n, f32), W1))
    wo = np.asarray(inputs["Wo"], f32).reshape(NL, C, C)
    w2 = np.asarray(inputs["W2"], f32).reshape(NL, C, C)

    wall = np.stack([wq, wk, wv, wo, w1, w2], axis=1)  # (NL, 6, C, C)
    wall = wall.reshape(NL * NW * P, C).astype(bf16)

    bcol = np.stack([bq.reshape(-1), bk.reshape(-1), b1f.reshape(-1)],
                    axis=1).astype(f32)  # (NL*C, 3)
    brow = np.stack([bv, np.asarray(inputs["bo"], f32),
                     np.asarray(inputs["b2"], f32)], axis=1)  # (NL, 3, C)
    brow = brow.reshape(NL * 3, C).astype(bf16)

    tri = np.zeros((P, P), f32)
    tri[np.tril_indices(P, -1)] = NEG  # tri[k, t] = NEG where t < k
    trib = tri.astype(bf16)
    identb = np.eye(P, dtype=bf16)

    wlm_pad = np.zeros((C, VPAD), f32)
    wlm_pad[:, :V] = np.asarray(inputs["Wlm"], f32)
    blm_pad = np.zeros((1, VPAD), f32)
    blm_pad[0, :V] = np.asarray(inputs["blm"], f32)
    has_blm = bool(np.any(blm_pad))

    common = {
        "tok_emb": tok_emb.astype(bf16),
        "pos": pos_emb.astype(bf16),
        "wall": wall,
        "bcol": bcol,
        "brow": brow,
        "ones": np.ones((1, 512), bf16),
        "identb": identb,
        "trib": trib,
    }
    in_maps = []
    for j in range(NCORE):
        b, q = divmod(j, NQ)
        im = dict(common)
        im["idx"] = np.ascontiguousarray(x[b].reshape(N, 1))
        im["wlm"] = np.ascontiguousarray(
            wlm_pad[:, q * VSH:(q + 1) * VSH]).astype(bf16)
        if has_blm:
            im["blm"] = np.ascontiguousarray(
                blm_pad[:, q * VSH:(q + 1) * VSH]).astype(bf16)
        in_maps.append(im)
    return in_maps, has_blm


def kernel(**inputs):
    in_maps, has_blm = _prep_inputs(inputs)
    key = ("nc", has_blm)
    if key not in _CACHE:
        _CACHE[key] = _build(has_blm)
    nc = _CACHE[key]
    res = bass_utils.run_bass_kernel_spmd(nc, in_maps,
                                          core_ids=list(range(NCORE)))
    logits = np.zeros((B, T, VPAD), np.float32)
    for j in range(NCORE):
        b, q = divmod(j, NQ)
        logits[b, :, q * VSH:(q + 1) * VSH] = \
            np.asarray(res.results[j]["logits"], np.float32)
    return logits[:, :, :V]


if __name__ == "__main__":
    pass


# revision 25
# speedup vs baseline: 1.5658x; 1.0124x over previous
"""Bass/Trainium2 kernel for nn_BigramLanguageModel (v3).

Sharding (8 NeuronCores, single SPMD launch, no collectives):
  - core j: batch b = j//4, vocab quarter q = j%4. Each core runs the
    3-layer transformer on its 1024-token batch (2-way data parallel)
    and computes logits[:, 12800*q : 12800*(q+1)] (4-way tensor
    parallel over the padded 51200 vocab). Host concatenates.
  - All matmul operands are bf16 (fp32 PSUM accumulation, fp32 residual
    stream h in SBUF). Logits leave the core as bf16 (halves the output
    DMA) and are upcast on host. rel-err budget 2e-2 >> bf16 ~2e-3.
  - LayerNorm affine is folded into the following projections
    host-side; 1/sqrt(HS) into Wk; q/k biases into the PSUM
    evacuations; b1 into the Relu evacuation; v/o/mlp2 biases ride as
    rank-1 PSUM-preload matmuls.
  - All 128x128 block transposes (LN outputs, LM-head h) run on the
    DMA engines (InstDmaTransposeAnt, 14ns/xbar-tile) instead of the
    PE array; the normalized activations are written token-major so
    one DMA transposes 2 tiles per call.
  - Softmax over the *query* axis in transposed score layout wT[k, t]:
    one Exp per (head, key-chunk); the denominator is a pass-through
    scalar_tensor_tensor on DVE (4x bf16 mode) with fused accum_out;
    1/denom folds into v rows via an ALU-divide on the Pool engine
    (gpsimd: SBUF-only work — it has no PSUM port).
  - Causal mask via bf16 (-80)-triangle PSUM-preload matmul.
  - Per-layer weights arrive as ONE packed DMA; LM-head weights
    prefetch at the start of the last layer; the LM head is fused into
    the last MLP tile loop so its GEMMs overlap the transformer tail.
"""

import sys

sys.path.insert(0, "/opt/trn_rl_repo")

import numpy as np

import concourse.bass as bass
import concourse.mybir as mybir
import concourse.tile as tile
from concourse import bacc
from concourse import bass_utils

F32 = mybir.dt.float32
BF16 = mybir.dt.bfloat16
I32 = mybir.dt.int32
AF = mybir.ActivationFunctionType
ALU = mybir.AluOpType

V, C, T, H, HS, NL, B = 50257, 384, 1024, 6, 64, 3, 2
P = 128
N = T                      # 1024 tokens per core (one batch)
NT = N // P                # 8 token chunks
NC3 = C // P               # 3 channel chunks
NCORE = 8
NQ = 4                     # vocab quarters
VPAD = 51200               # padded vocab (4 * 12800)
VSH = VPAD // NQ           # 12800 vocab columns per core
KC = T // P                # 8 key chunks
TB = T // 512              # 2 query blocks of 512
NEG = -80.0                # mask bias (exp(-80) ~ 1.8e-35)
NW = 18                    # packed weight tiles per layer (6 mats x 3 chunks)

_CACHE: dict = {}
PHASES: list = []


def _mark(nc, label):
    PHASES.append((label, int(nc.next_id())))


class _EvacSplit:
    """Round-robin PSUM->SBUF evacuation copies over DVE / ACT.
    (gpsimd has no PSUM port, so Pool is not in this rotation.)"""

    def __init__(self, nc):
        self.nc = nc
        self.i = 0

    def copy(self, out, in_):
        self.i += 1
        if self.i % 2 == 0:
            self.nc.vector.tensor_copy(out, in_)
        else:
            self.nc.scalar.copy(out, in_)


def _build(has_blm: bool):
    nc = bacc.Bacc("TRN2", target_bir_lowering=False, debug=False)

    d_idx = nc.dram_tensor("idx", [N, 1], I32, kind="ExternalInput").ap()
    d_tok = nc.dram_tensor("tok_emb", [V, C], BF16, kind="ExternalInput").ap()
    d_pos = nc.dram_tensor("pos", [N, C], BF16, kind="ExternalInput").ap()
    # packed per-layer weights: rows (l*NW + k)*P + p, k = mat*3 + chunk
    d_wall = nc.dram_tensor("wall", [NL * NW * P, C], BF16,
                            kind="ExternalInput").ap()
    # packed per-partition bias columns: [bq | bk | b1] per layer
    d_bcol = nc.dram_tensor("bcol", [NL * C, 3], F32,
                            kind="ExternalInput").ap()
    # packed bias rows [bv ; bo ; b2] per layer
    d_brow = nc.dram_tensor("brow", [NL * 3, C], BF16,
                            kind="ExternalInput").ap()
    d_ones = nc.dram_tensor("ones", [1, 512], BF16, kind="ExternalInput").ap()
    d_identb = nc.dram_tensor("identb", [P, P], BF16,
                              kind="ExternalInput").ap()
    d_trib = nc.dram_tensor("trib", [P, P], BF16, kind="ExternalInput").ap()
    d_wlm = nc.dram_tensor("wlm", [C, VSH], BF16, kind="ExternalInput").ap()
    if has_blm:
        d_blm = nc.dram_tensor("blm", [1, VSH], BF16,
                               kind="ExternalInput").ap()
    d_out = nc.dram_tensor("logits", [N, VSH], BF16,
                           kind="ExternalOutput").ap()

    with tile.TileContext(nc) as tc:
        _emit(nc, tc, locals(), has_blm)
    nc.compile()
    return nc


def _emit(nc, tc, d, has_blm):
    from contextlib import ExitStack

    with ExitStack() as ctx:
        hpool = ctx.enter_context(tc.tile_pool(name="hpool", bufs=NT))
        pers = ctx.enter_context(tc.tile_pool(name="pers", bufs=1))
        spool = ctx.enter_context(tc.tile_pool(name="spool", bufs=8))

        # ------------- embedding gather (DMAs issued first) -------------
        _mark(nc, "embed")
        h = []  # 8 residual-stream tiles (128, 384) fp32
        idx_all = pers.tile([P, NT], I32, name="idx", tag="idx")
        nc.sync.dma_start(
            idx_all[:],
            bass.AP(tensor=d["d_idx"].tensor, offset=d["d_idx"].offset,
                    ap=[[1, P], [P, NT]]))
        posw = pers.tile([P, NT * C], BF16, name="pos", tag="pos")
        nc.sync.dma_start(
            posw[:],
            bass.AP(tensor=d["d_pos"].tensor, offset=d["d_pos"].offset,
                    ap=[[C, P], [P * C, NT], [1, C]]))

        # ------------- constants -------------
        ones = pers.tile([1, 512], BF16, name="ones", tag="ones")
        identb = pers.tile([P, P], BF16, name="identb", tag="identb")
        trib = pers.tile([P, P], BF16, name="trib", tag="trib")
        eps = pers.tile([P, 1], F32, name="eps", tag="eps")
        nc.sync.dma_start(ones[:], d["d_ones"][:])
        nc.sync.dma_start(identb[:], d["d_identb"][:])
        nc.sync.dma_start(trib[:], d["d_trib"][:])
        nc.vector.memset(eps[:], 1e-5)

        with tc.tile_pool(name="epool", bufs=3) as epool:
            for i in range(NT):
                emb = epool.tile([P, C], BF16, name="emb", tag="emb")
                nc.gpsimd.indirect_dma_start(
                    out=emb[:], out_offset=None, in_=d["d_tok"][:],
                    in_offset=bass.IndirectOffsetOnAxis(
                        ap=idx_all[:, i:i + 1], axis=0),
                )
                h_i = hpool.tile([P, C], F32, name="h", tag="h")
                nc.vector.tensor_add(h_i[:], emb[:],
                                     posw[:, i * C:(i + 1) * C])
                h.append(h_i)

        ev = _EvacSplit(nc)

        # LM-head weights: persistent tiles, prefetched in small chunks
        # from layer 1 onward so the transfers never head-of-line block
        # the LN transpose DMAs on the (serialized) DMA engines.
        lmpool = ctx.enter_context(tc.tile_pool(name="lmpool", bufs=1))
        wlm = [lmpool.tile([P, VSH], BF16, name=f"wlm{c}", tag=f"wlm{c}")
               for c in range(NC3)]
        blm = None
        if has_blm:
            blm = lmpool.tile([1, VSH], BF16, name="blm", tag="blm")

        # ------------- layers -------------
        for l in range(NL):
            with ExitStack() as lctx:
                wpool = lctx.enter_context(
                    tc.tile_pool(name=f"wpool{l}", bufs=1))
                wall = wpool.tile([P, NW * C], BF16, name="wall", tag="wall")
                r0 = l * NW * P
                for half in range(2):
                    nc.sync.dma_start(
                        wall[:, half * 9 * C:(half + 1) * 9 * C],
                        bass.AP(tensor=d["d_wall"].tensor,
                                offset=(d["d_wall"].offset
                                        + (r0 + half * 9 * P) * C),
                                ap=[[C, P], [P * C, 9], [1, C]]))

                def wslice(mat, c):
                    k = mat * 3 + c
                    return wall[:, k * C:(k + 1) * C]

                wq = [wslice(0, c) for c in range(NC3)]
                wk = [wslice(1, c) for c in range(NC3)]
                wv = [wslice(2, c) for c in range(NC3)]
                wo = [wslice(3, c) for c in range(NC3)]
                w1 = [wslice(4, c) for c in range(NC3)]
                w2 = [wslice(5, c) for c in range(NC3)]

                bcol = wpool.tile([P, NC3 * 3], F32, name="bcol", tag="bcol")
                nc.sync.dma_start(
                    bcol[:],
                    bass.AP(tensor=d["d_bcol"].tensor,
                            offset=d["d_bcol"].offset + l * C * 3,
                            ap=[[3, P], [P * 3, NC3], [1, 3]]))
                bqkt = [bcol[:, 3 * c:3 * c + 2] for c in range(NC3)]
                b1t = [bcol[:, 3 * c + 2:3 * c + 3] for c in range(NC3)]
                brow = wpool.tile([1, 3 * C], BF16, name="brow", tag="brow")
                nc.sync.dma_start(
                    brow[:],
                    bass.AP(tensor=d["d_brow"].tensor,
                            offset=d["d_brow"].offset + l * 3 * C,
                            ap=[[3 * C, 1], [1, 3 * C]]))
                bv = brow[:, 0:C]
                bo = brow[:, C:2 * C]
                b2 = brow[:, 2 * C:3 * C]

                if l == 1:
                    # prefetch LM-head weights in 12 small chunks
                    for c in range(NC3):
                        for q4 in range(4):
                            nc.sync.dma_start(
                                wlm[c][:, q4 * 3200:(q4 + 1) * 3200],
                                d["d_wlm"][c * P:(c + 1) * P,
                                           q4 * 3200:(q4 + 1) * 3200])
                    if has_blm:
                        nc.sync.dma_start(blm[:], d["d_blm"][:])

                with ExitStack() as actx:
                    attpool = actx.enter_context(
                        tc.tile_pool(name=f"attpool{l}", bufs=1))
                    attT = [attpool.tile([P, N], BF16, name=f"attT{c}",
                                         tag=f"attT{c}")
                            for c in range(NC3)]
                    with ExitStack() as qctx:
                        atp = qctx.enter_context(
                            tc.tile_pool(name=f"atp{l}", bufs=1))
                        _mark(nc, f"L{l}.ln1")
                        at = _layernorm_transposed(
                            nc, tc, h, eps, atp, spool, f"a{l}")
                        _mark(nc, f"L{l}.v")

                        vpool = qctx.enter_context(
                            tc.tile_pool(name=f"vpool{l}", bufs=NT))
                        psc = qctx.enter_context(tc.tile_pool(
                            name=f"psc{l}", bufs=2, space="PSUM"))
                        psa = qctx.enter_context(tc.tile_pool(
                            name=f"psa{l}", bufs=2, space="PSUM"))
                        v = []
                        for i in range(NT):
                            ps = psc.tile([P, C], F32, name="psc", tag="psc")
                            nc.tensor.matmul(ps[:], ones[:, :P], bv,
                                             start=True, stop=False)
                            for c in range(NC3):
                                nc.tensor.matmul(
                                    ps[:], at(c, i), wv[c], start=False,
                                    stop=(c == NC3 - 1))
                            v_i = vpool.tile([P, C], BF16, name="v", tag="v")
                            ev.copy(v_i[:], ps[:])
                            v.append(v_i)

                        qkpool = qctx.enter_context(
                            tc.tile_pool(name=f"qkpool{l}", bufs=2))
                        ppool = qctx.enter_context(
                            tc.tile_pool(name=f"ppool{l}", bufs=4))
                        vspool = qctx.enter_context(
                            tc.tile_pool(name=f"vspool{l}", bufs=6))
                        _mark(nc, f"L{l}.attn")

                        def build_qk(m):
                            qT_m = qkpool.tile([P, N], BF16, name="qT",
                                               tag="qT")
                            kT_m = qkpool.tile([P, N], BF16, name="kT",
                                               tag="kT")
                            for dst, wmat, bc in (
                                    (qT_m, wq, bqkt[m][:, 0:1]),
                                    (kT_m, wk, bqkt[m][:, 1:2])):
                                for t4 in range(N // 512):
                                    ps = psc.tile([P, 512], F32, name="psc",
                                                  tag="psc")
                                    for c in range(NC3):
                                        nc.tensor.matmul(
                                            ps[:],
                                            wmat[c][:, m * P:(m + 1) * P],
                                            at(c, t4, blk=True),
                                            start=(c == 0),
                                            stop=(c == NC3 - 1))
                                    nc.vector.scalar_tensor_tensor(
                                        dst[:, t4 * 512:(t4 + 1) * 512],
                                        ps[:], bc,
                                        dst[:, t4 * 512:(t4 + 1) * 512],
                                        op0=ALU.add, op1=ALU.bypass)
                            return qT_m, kT_m

                        qk_next = build_qk(0)
                        for m in range(NC3):
                            qT_m, kT_m = qk_next
                            qk_next = None
                            _attention_m(
                                nc, l, m, qT_m, kT_m, v, attT, trib,
                                identb, ppool, vspool, spool, psc, psa, ev,
                                mid=(lambda mm=m: build_qk(mm + 1))
                                if m + 1 < NC3 else None)
                            if m + 1 < NC3:
                                qk_next = _attention_m.qk_built

                    _mark(nc, f"L{l}.proj")
                    with tc.tile_pool(name=f"pso{l}", bufs=2,
                                      space="PSUM") as pso:
                        for i in range(NT):
                            ps = pso.tile([P, C], F32, name="pmm", tag="pmm")
                            nc.tensor.matmul(ps[:], ones[:, :P], bo,
                                             start=True, stop=False)
                            for c in range(NC3):
                                nc.tensor.matmul(
                                    ps[:], attT[c][:, i * P:(i + 1) * P],
                                    wo[c], start=False, stop=(c == NC3 - 1))
                            nc.vector.tensor_add(h[i][:], h[i][:], ps[:])

                # --- LN2 + MLP (+ fused LM head on the last layer) ---
                _mark(nc, f"L{l}.mlp")
                with ExitStack() as mctx:
                    atp2 = mctx.enter_context(
                        tc.tile_pool(name=f"atp2{l}", bufs=1))
                    m1pool = mctx.enter_context(
                        tc.tile_pool(name=f"m1pool{l}", bufs=3))
                    a2t = _layernorm_transposed(
                        nc, tc, h, eps, atp2, spool, f"b{l}")
                    if l < NL - 1:
                        psm = mctx.enter_context(tc.tile_pool(
                            name=f"psm{l}", bufs=4, space="PSUM"))
                        ps_m1 = psm
                    else:
                        pstm = mctx.enter_context(tc.tile_pool(
                            name="pstm", bufs=2, space="PSUM"))
                        pslm = mctx.enter_context(tc.tile_pool(
                            name="pslm", bufs=6, space="PSUM"))
                        ps_m1 = pslm
                    m1T = [m1pool.tile([P, N], BF16, name="m1T", tag="m1T")
                           for _ in range(NC3)]
                    for cm in range(NC3):
                        for t4 in range(N // 512):
                            ps = ps_m1.tile([P, 512], F32, name="plm",
                                            tag="plm")
                            for c in range(NC3):
                                nc.tensor.matmul(
                                    ps[:], w1[c][:, cm * P:(cm + 1) * P],
                                    a2t(c, t4, blk=True),
                                    start=(c == 0), stop=(c == NC3 - 1))
                            nc.scalar.activation(
                                m1T[cm][:, t4 * 512:(t4 + 1) * 512],
                                ps[:], AF.Relu, bias=b1t[cm][:, 0:1])

                    if l < NL - 1:
                        for i in range(NT):
                            ps = psm.tile([P, C], F32, name="pmm", tag="pmm")
                            nc.tensor.matmul(ps[:], ones[:, :P], b2,
                                             start=True, stop=False)
                            for c in range(NC3):
                                nc.tensor.matmul(
                                    ps[:], m1T[c][:, i * P:(i + 1) * P],
                                    w2[c], start=False, stop=(c == NC3 - 1))
                            nc.vector.tensor_add(h[i][:], h[i][:], ps[:])
                    else:
                        _mark(nc, "lmhead")
                        hbpool = mctx.enter_context(
                            tc.tile_pool(name="hbpool", bufs=2))
                        opool = mctx.enter_context(
                            tc.tile_pool(name="opool", bufs=2))
                        for i in range(NT):
                            ps = pstm.tile([P, C], F32, name="pmm2",
                                           tag="pmm2")
                            nc.tensor.matmul(ps[:], ones[:, :P], b2,
                                             start=True, stop=False)
                            for c in range(NC3):
                                nc.tensor.matmul(
                                    ps[:], m1T[c][:, i * P:(i + 1) * P],
                                    w2[c], start=False, stop=(c == NC3 - 1))
                            hb = hbpool.tile([P, C], BF16, name="hb",
                                             tag="hb")
                            nc.vector.tensor_add(hb[:], h[i][:], ps[:])
                            # transpose hb -> hT (3 chunks) on the DMA xbar
                            hT = hbpool.tile([P, C], BF16, name="hT",
                                             tag="hT")
                            hT_ap = bass.AP(tensor=hT.tensor,
                                            offset=hT.offset,
                                            ap=[hT.ap[0], [P, NC3], [1, P]])
                            nc.scalar.dma_start_transpose(hT_ap, hb[:])
                            # 25 x 512-wide logits tiles for this token
                            # chunk, stationary operand outermost in groups
                            # of 3 (1 Ldweights per (group, c))
                            ost = opool.tile([P, VSH], BF16, name="ost",
                                             tag="ost")
                            nvg = VSH // 512
                            for g0 in range(0, nvg, 3):
                                vgs = range(g0, min(g0 + 3, nvg))
                                pvs = {}
                                for vg in vgs:
                                    pvs[vg] = pslm.tile([P, 512], F32,
                                                        name="plm",
                                                        tag="plm")
                                    if has_blm:
                                        nc.tensor.matmul(
                                            pvs[vg][:], ones[:, :P],
                                            blm[:, vg * 512:(vg + 1) * 512],
                                            start=True, stop=False)
                                for c in range(NC3):
                                    for vg in vgs:
                                        nc.tensor.matmul(
                                            pvs[vg][:],
                                            hT[:, c * P:(c + 1) * P],
                                            wlm[c][:,
                                                   vg * 512:(vg + 1) * 512],
                                            start=(c == 0 and not has_blm),
                                            stop=(c == NC3 - 1))
                                for vg in vgs:
                                    ev.copy(ost[:, vg * 512:(vg + 1) * 512],
                                            pvs[vg][:])
                            nc.sync.dma_start(
                                d["d_out"][i * P:(i + 1) * P, :], ost[:])


def _layernorm_transposed(nc, tc, h, eps, atpool, spool, label):
    """LN (affine folded into weights host-side); the normalized
    activations are written token-major into aW and transposed by the
    DMA xbar into aT (24 x 128x128 blocks at stride 128). Returns an
    accessor at(c, i) -> [128,128] chunk-c block of token tile i, or
    at(c, t4, blk=True) -> [128, 512] strided 4-block moving operand."""
    mv = spool.tile([P, 2 * NT], F32, name="mv", tag="mv")
    for i in range(NT):
        st = spool.tile([P, 6], F32, name="st", tag="st")
        nc.vector.bn_stats(st[:], h[i][:])
        nc.vector.bn_aggr(mv[:, 2 * i:2 * i + 2], st[:])
    std = spool.tile([P, NT], F32, name="std", tag="std")
    rstd = spool.tile([P, NT], F32, name="rstd", tag="rstd")
    nmr = spool.tile([P, NT], F32, name="nmr", tag="nmr")
    for g in range(0, NT, 4):
        nc.scalar.activation(std[:, g:g + 4], mv[:, 2 * g + 1:2 * g + 8:2],
                             AF.Sqrt, bias=eps[:, :1])
        nc.vector.reciprocal(rstd[:, g:g + 4], std[:, g:g + 4])
        nc.vector.scalar_tensor_tensor(nmr[:, g:g + 4],
                                       mv[:, 2 * g:2 * g + 8:2], -1.0,
                                       rstd[:, g:g + 4],
                                       op0=ALU.mult, op1=ALU.mult)
    aW = atpool.tile([P, NT * C], BF16, name=f"aW{label}", tag="aW")
    aTw = atpool.tile([P, NT * C], BF16, name=f"aT{label}", tag="aT")
    for i in range(NT):
        nc.vector.tensor_scalar(aW[:, i * C:(i + 1) * C], h[i][:],
                                rstd[:, i:i + 1], nmr[:, i:i + 1],
                                op0=ALU.mult, op1=ALU.add)
    for i2 in range(NT // 2):
        out_ap = bass.AP(tensor=aTw.tensor,
                         offset=aTw.offset + i2 * 2 * C,
                         ap=[aTw.ap[0], [P, 2 * NC3], [1, P]])
        nc.scalar.dma_start_transpose(out_ap, aW[:, i2 * 2 * C:
                                                 (i2 + 1) * 2 * C])

    def at(c, i, blk=False):
        if blk:  # 512 tokens: 4 tiles of 128 at stride C
            return bass.AP(tensor=aTw.tensor,
                           offset=aTw.offset + i * 4 * C + c * P,
                           ap=[aTw.ap[0], [C, 4], [1, P]])
        return bass.AP(tensor=aTw.tensor,
                       offset=aTw.offset + i * C + c * P,
                       ap=[aTw.ap[0], [1, P]])

    return at


def _attention_m(nc, l, m, qT_m, kT_m, v, attT, trib, identb,
                 ppool, vspool, spool, psc, psa, ev, mid=None):
    """Scores + query-axis softmax + p@v for heads (2m, 2m+1).

    Scores for one (head, key-chunk) land in a (128, 1024) two-bank PSUM
    tile; one Exp covers the causally-valid range [128*kc : 1024). The
    denominator comes from a pass-through STT on DVE (4x bf16) with
    fused accum_out; v rows are divided by it on the Pool engine. p@v
    accumulates in (64, 1024) PSUM tiles, two heads in separate free
    halves, pipelined 2 key-chunks behind the scores."""
    d0 = spool.tile([P, 16], F32, name="d0", tag="d0")
    dinv = spool.tile([P, 16], F32, name="dinv", tag="dinv")
    dsc = spool.tile([P, T], BF16, name="dsc", tag="dsc")

    att_ps = {tb: psa.tile([64, 1024], F32, name="patt", tag="patt")
              for tb in range(TB)}
    pending = []

    for kc in range(KC):
        p_kc = ppool.tile([P, 2 * T], BF16, name="p", tag="p")
        lo_kc = 128 * kc
        w_kc = T - lo_kc
        diag_tb = kc // (512 // P)
        for hh in range(2):
            pp = psc.tile([P, T], F32, name="psc", tag="psc")
            nc.tensor.matmul(pp[:, lo_kc:lo_kc + P], identb[:], trib[:],
                             start=True, stop=False)
            for tb in range(TB):
                lo = 128 * kc - 512 * tb
                if lo >= 512:
                    continue
                lo = max(lo, 0)
                nc.tensor.matmul(
                    pp[:, tb * 512 + lo:(tb + 1) * 512],
                    kT_m[64 * hh:64 * hh + 64, lo_kc:lo_kc + P],
                    qT_m[64 * hh:64 * hh + 64,
                         tb * 512 + lo:(tb + 1) * 512],
                    start=(tb != diag_tb), stop=(tb == TB - 1))
            nc.scalar.activation(
                p_kc[:, hh * T + lo_kc:(hh + 1) * T],
                pp[:, lo_kc:T], AF.Exp)
            # denominator: pass-through tensor_scalar (4x bf16 on DVE)
            # with fused row-sum accum
            nc.vector.tensor_scalar(
                dsc[:, :w_kc], p_kc[:, hh * T + lo_kc:(hh + 1) * T],
                0.0, 0.0, op0=ALU.add, op1=ALU.add,
                accum_out=d0[:, 8 * hh + kc:8 * hh + kc + 1])

        nc.vector.reciprocal(dinv[:, kc::8], d0[:, kc::8])
        vs = vspool.tile([P, P], BF16, name="vs", tag="vs")
        for hh in range(2):
            vslice = v[kc][:, m * P + 64 * hh:m * P + 64 * hh + 64]
            nc.vector.scalar_tensor_tensor(
                vs[:, 64 * hh:64 * hh + 64], vslice,
                dinv[:, 8 * hh + kc:8 * hh + kc + 1], vslice,
                op0=ALU.mult, op1=ALU.bypass)
        pending.append((kc, p_kc, vs))
        if len(pending) > 2:
            _emit_att(nc, attT, att_ps, m, *pending.pop(0))

    if mid is not None:
        _attention_m.qk_built = mid()
    while pending:
        _emit_att(nc, attT, att_ps, m, *pending.pop(0))
    for hh in range(2):
        if hh == 0:
            nc.vector.tensor_copy(
                attT[m][0:64, 512:1024], att_ps[1][:, 0:512])
        else:
            nc.scalar.copy(
                attT[m][64:128, 512:1024], att_ps[1][:, 512:1024])


def _emit_att(nc, attT, att_ps, m, kc, p_kc, vs):
    for tb in range(TB):
        lo = 128 * kc - 512 * tb
        if lo >= 512:
            continue
        lo = max(lo, 0)
        last = (kc == (3 if tb == 0 else KC - 1))
        for hh in range(2):
            nc.tensor.matmul(
                att_ps[tb][:, hh * 512 + lo:(hh + 1) * 512],
                vs[:, 64 * hh:64 * hh + 64],
                p_kc[:, hh * T + tb * 512 + lo:hh * T + (tb + 1) * 512],
                start=(kc == 0), stop=last, skip_group_check=True)
    if kc == 3:
        nc.vector.tensor_copy(attT[m][0:64, 0:512], att_ps[0][:, 0:512])
        nc.scalar.copy(attT[m][64:128, 0:512], att_ps[0][:, 512:1024])


# ---------------------------------------------------------------------------
# host side
# ---------------------------------------------------------------------------

def _prep_inputs(inputs):
    import ml_dtypes
    f32 = np.float32
    bf16 = ml_dtypes.bfloat16
    tok_emb = np.asarray(inputs["tok_emb"], f32)
    pos_emb = np.asarray(inputs["pos_emb"], f32)
    x = np.asarray(inputs["x"]).astype(np.int32)  # (B, T)

    def fold_qkv(W, bias, g, b_ln, extra=1.0):
        Wf = np.transpose(np.asarray(W, f32), (0, 2, 1, 3)).reshape(NL, C, C)
        bf = (np.asarray(bias, f32).reshape(NL, C)
              + np.einsum("lc,lcd->ld", np.asarray(b_ln, f32), Wf))
        Wg = Wf * np.asarray(g, f32)[:, :, None]
        return (Wg * extra), (bf * extra)

    g1, b1n = inputs["ln1_g"], inputs["ln1_b"]
    g2, b2n = inputs["ln2_g"], inputs["ln2_b"]
    wq, bq = fold_qkv(inputs["Wq"], inputs["bq"], g1, b1n)
    wk, bk = fold_qkv(inputs["Wk"], inputs["bk"], g1, b1n, extra=HS ** -0.5)
    wv, bv = fold_qkv(inputs["Wv"], inputs["bv"], g1, b1n)

    W1 = np.asarray(inputs["W1"], f32)
    w1 = W1 * np.asarray(g2, f32)[:, :, None]
    b1f = (np.asarray(inputs["b1"], f32)
           + np.einsum("lc,lcd->ld", np.asarray(b2n, f32), W1))
    wo = np.asarray(inputs["Wo"], f32).reshape(NL, C, C)
    w2 = np.asarray(inputs["W2"], f32).reshape(NL, C, C)

    wall = np.stack([wq, wk, wv, wo, w1, w2], axis=1)  # (NL, 6, C, C)
    wall = wall.reshape(NL * NW * P, C).astype(bf16)

    bcol = np.stack([bq.reshape(-1), bk.reshape(-1), b1f.reshape(-1)],
                    axis=1).astype(f32)  # (NL*C, 3)
    brow = np.stack([bv, np.asarray(inputs["bo"], f32),
                     np.asarray(inputs["b2"], f32)], axis=1)  # (NL, 3, C)
    brow = brow.reshape(NL * 3, C).astype(bf16)

    tri = np.zeros((P, P), f32)
    tri[np.tril_indices(P, -1)] = NEG  # tri[k, t] = NEG where t < k
    trib = tri.astype(bf16)
    identb = np.eye(P, dtype=bf16)

    wlm_pad = np.zeros((C, VPAD), f32)
    wlm_pad[:, :V] = np.asarray(inputs["Wlm"], f32)
    blm_pad = np.zeros((1, VPAD), f32)
    blm_pad[0, :V] = np.asarray(inputs["blm"], f32)
    has_blm = bool(np.any(blm_pad))

    common = {
        "tok_emb": tok_emb.astype(bf16),
        "pos": pos_emb.astype(bf16),
        "wall": wall,
        "bcol": bcol,
        "brow": brow,
        "ones": np.ones((1, 512), bf16),
        "identb": identb,
        "trib": trib,
    }
    in_maps = []
    for j in range(NCORE):
        b, q = divmod(j, NQ)
        im = dict(common)
        im["idx"] = np.ascontiguousarray(x[b].reshape(N, 1))
        im["wlm"] = np.ascontiguousarray(
            wlm_pad[:, q * VSH:(q + 1) * VSH]).astype(bf16)
        if has_blm:
            im["blm"] = np.ascontiguousarray(
                blm_pad[:, q * VSH:(q + 1) * VSH]).astype(bf16)
        in_maps.append(im)
    return in_maps, has_blm


def kernel(**inputs):
    in_maps, has_blm = _prep_inputs(inputs)
    key = ("nc", has_blm)
    if key not in _CACHE:
        _CACHE[key] = _build(has_blm)
    nc = _CACHE[key]
    res = bass_utils.run_bass_kernel_spmd(nc, in_maps,
                                          core_ids=list(range(NCORE)))
    logits = np.zeros((B, T, VPAD), np.float32)
    for j in range(NCORE):
        b, q = divmod(j, NQ)
        logits[b, :, q * VSH:(q + 1) * VSH] = \
            np.asarray(res.results[j]["logits"], np.float32)
    return logits[:, :, :V]


if __name__ == "__main__":
    pass
